# revision 11
# baseline (speedup 1.0000x reference)
"""Causal+padding-mask attention kernel for 8 Trainium2 NeuronCores.

Problem: B=4, H=16, S=2048, D=64 fp32 attention that returns BOTH the
attention output [B,H,S,D] and the normalized attention weights
[B,H,S,S] (the reference's softmax over masked scores).

Reference mask quirk: scores are masked where (future + pad) == 1, i.e.
future XOR pad. A "future" position whose key padding bit is 1 is
UNMASKED (raw score kept).

Sharding: 64 (b,h) pairs over 8 cores -> core c handles batch b=c//2,
heads h = (c%2)*8 .. +8. No inter-core communication.

Per-core kernel design (T-layout primary):
  - Q,K loaded [128q,64d] tiles, PE-transposed to qT,kT [64d, 2048].
  - S_T[k,q] = K @ Q^T computed with k on partitions via
    matmul(lhsT=kT slice, rhs=qT slice) in float32r (full rate at N=512).
  - Masking: in T layout the mask value depends (off-diagonal) only on k
    = the partition index, so it folds into the per-partition bias of the
    ACT exp: p = Exp(0.125*S_T + bias[k]). bias_a[k] = -1e9*pad[k] for
    "past" tiles, bias_b[k] = -1e9*(1-pad[k]) for "future" tiles. Only
    the 16 diagonal 128x128 tiles need an elementwise fix (DVE, tiny).
    No row-max subtraction is needed: scaled scores are ~N(0,1), max ~6,
    exp stays in fp32 range.
  - PV: matmul(lhsT=Vp[kc] [128k,65], rhs=p [128k,512q]) accumulates
    out_T[d,q] AND the softmax denominator (ones column appended to V).
  - Weights out: PE-transpose p tiles back to [q,k], normalize by 1/denom
    (per-partition in q layout) fused into the PSUM->SBUF copy on DVE,
    DMA [128,2048] fp32 rows to HBM.
  - Output out[q,d]: small PE transposes of out_T + per-partition scale.
"""

import os
from contextlib import ExitStack

import numpy as np

import concourse.bacc as bacc
import concourse.bass as bass
import concourse.mybir as mybir
import concourse.tile as tile
from concourse.bass_utils import run_bass_kernel_spmd
from concourse.masks import make_identity

B, H, S, D = 4, 16, 2048, 64
NCORES = 8
CORES_PER_B = NCORES // B          # 2
HPC = H // CORES_PER_B             # 8 heads per core
NT = S // 128                      # 16 k/q tiles of 128
F32 = mybir.dt.float32
F32R = mybir.dt.float32r
EXP = mybir.ActivationFunctionType.Exp
NEG = -1.0e9

# float32r transposes are 1.33x faster on PE; they round the weights
# through ~tf32 precision. Flip to False if accuracy is too tight.
F32R_TRANSPOSE = True


def build(nP=HPC, repeat=1):
    """Build the per-core Bass program processing nP (b,h) pairs."""
    nc = bacc.Bacc("TRN2", target_bir_lowering=False, debug=False)

    q_d = nc.dram_tensor("q", [nP, S, D], F32, kind="ExternalInput")
    k_d = nc.dram_tensor("k", [nP, S, D], F32, kind="ExternalInput")
    v_d = nc.dram_tensor("v", [nP, S, D], F32, kind="ExternalInput")
    ba_d = nc.dram_tensor("biasa", [128, NT], F32, kind="ExternalInput")
    bb_d = nc.dram_tensor("biasb", [128, NT], F32, kind="ExternalInput")
    dg_d = nc.dram_tensor("diag", [128, S], F32, kind="ExternalInput")
    w_d = nc.dram_tensor("w", [nP, S, S], F32, kind="ExternalOutput")
    o_d = nc.dram_tensor("o", [nP, S, D], F32, kind="ExternalOutput")

    with tile.TileContext(nc) as tc, ExitStack() as ctx:
        const = ctx.enter_context(tc.tile_pool(name="const", bufs=1))
        ident = const.tile([128, 128], F32)
        make_identity(nc, ident[:])
        ident_r = const.tile([128, 128], F32R)
        nc.scalar.copy(ident_r[:], ident[:])
        ba = const.tile([128, NT], F32)
        nc.sync.dma_start(ba[:], ba_d.ap())
        bb = const.tile([128, NT], F32)
        nc.sync.dma_start(bb[:], bb_d.ap())
        dg = const.tile([128, S], F32)
        nc.sync.dma_start(dg[:], dg_d.ap())

        natp = ctx.enter_context(tc.tile_pool(name="nat", bufs=2))
        ktp = ctx.enter_context(tc.tile_pool(name="kt", bufs=2))
        vp = ctx.enter_context(tc.tile_pool(name="v", bufs=2))
        pp = ctx.enter_context(tc.tile_pool(name="p", bufs=34))
        wsb = ctx.enter_context(tc.tile_pool(name="wsb", bufs=3))
        ovp = ctx.enter_context(tc.tile_pool(name="ov", bufs=2))
        osb = ctx.enter_context(tc.tile_pool(name="osb", bufs=2))
        csb = ctx.enter_context(tc.tile_pool(name="c", bufs=2))
        dtp = ctx.enter_context(tc.tile_pool(name="dtmp", bufs=2))

        sps = ctx.enter_context(tc.tile_pool(name="sps", bufs=3, space="PSUM"))
        pvps = ctx.enter_context(tc.tile_pool(name="pvps", bufs=2, space="PSUM"))
        miscps = ctx.enter_context(tc.tile_pool(name="mps", bufs=3, space="PSUM"))

        PVLAG = 2

        def body():
            pending = []

            def drain():
                if pending:
                    pending.pop(0)()

            for n in range(nP):
                # ---- load Q,K,V; build qT,kT [64, S] and Vp [128, 16*65]
                q_nat = natp.tile([128, NT * D], F32, tag="qnat")
                k_nat = natp.tile([128, NT * D], F32, tag="knat")
                nc.sync.dma_start(
                    q_nat[:].rearrange("p (t d) -> p t d", d=D),
                    q_d.ap()[n].rearrange("(t p) d -> p t d", p=128),
                )
                nc.sync.dma_start(
                    k_nat[:].rearrange("p (t d) -> p t d", d=D),
                    k_d.ap()[n].rearrange("(t p) d -> p t d", p=128),
                )
                qT = ktp.tile([64, S], F32R, tag="qT")
                kT = ktp.tile([64, S], F32R, tag="kT")
                for src, dst in ((q_nat, qT), (k_nat, kT)):
                    for g in range(4):
                        tp = miscps.tile([64, 512], F32, tag="m", name="tp")
                        for u in range(4):
                            t = g * 4 + u
                            nc.tensor.transpose(
                                tp[:, u * 128:(u + 1) * 128],
                                src[:, t * D:(t + 1) * D],
                                ident[:, :128],
                            )
                        nc.scalar.copy(dst[:, g * 512:(g + 1) * 512], tp[:])
                v_sb = vp.tile([128, NT * 65], F32)
                vv = v_sb[:].rearrange("p (t c) -> p t c", c=65)
                nc.gpsimd.memset(vv[:, :, 64], 1.0)
                nc.sync.dma_start(
                    vv[:, :, 0:64],
                    v_d.ap()[n].rearrange("(t p) d -> p t d", p=128),
                )
                v_r = vp.tile([128, NT * 65], F32R, tag="vr")
                nc.scalar.copy(v_r[:], v_sb[:])

                for qc in range(4):
                    q0 = qc * 512
                    pv = pvps.tile([65, 512], F32)
                    p_tiles = []
                    pv_jobs = []
                    for kc in range(NT):
                        s_ps = sps.tile([128, 512], F32, tag="s")
                        nc.tensor.matmul(
                            s_ps[:],
                            lhsT=kT[:, kc * 128:(kc + 1) * 128],
                            rhs=qT[:, q0:q0 + 512],
                            start=True,
                            stop=True,
                        )
                        p_sb = pp.tile([128, 512], F32R, tag="p")
                        jstar = kc - 4 * qc
                        if 0 <= jstar < 4:
                            lo, hi = jstar * 128, jstar * 128 + 128
                            if lo > 0:
                                nc.scalar.activation(
                                    p_sb[:, :lo], s_ps[:, :lo], EXP,
                                    bias=bb[:, kc:kc + 1], scale=0.125,
                                )
                            if hi < 512:
                                nc.scalar.activation(
                                    p_sb[:, hi:], s_ps[:, hi:], EXP,
                                    bias=ba[:, kc:kc + 1], scale=0.125,
                                )
                            tmp = dtp.tile([128, 128], F32, tag="d")
                            nc.vector.tensor_add(
                                tmp[:], s_ps[:, lo:hi], dg[:, kc * 128:(kc + 1) * 128]
                            )
                            nc.scalar.activation(
                                p_sb[:, lo:hi], tmp[:], EXP, scale=0.125
                            )
                        else:
                            bias = ba if kc < 4 * qc else bb
                            nc.scalar.activation(
                                p_sb[:], s_ps[:], EXP,
                                bias=bias[:, kc:kc + 1], scale=0.125,
                            )
                        p_tiles.append(p_sb)

                        def mk_pv(kc=kc, p_sb=p_sb, pv=pv, v_r=v_r):
                            def f():
                                nc.tensor.matmul(
                                    pv[:],
                                    lhsT=v_r[:, kc * 65:(kc + 1) * 65],
                                    rhs=p_sb[:],
                                    start=(kc == 0),
                                    stop=(kc == NT - 1),
                                )
                            return f

                        pv_jobs.append(mk_pv())
                        if kc >= PVLAG:
                            pv_jobs[kc - PVLAG]()
                        drain()
                    for kc in range(NT - PVLAG, NT):
                        pv_jobs[kc]()
                    drain()

                    # defer epilogue + weights phase into the next q-block's
                    # kc loop so PE/ACT/DVE stay continuously fed
                    state = {}

                    def mk_epi(n=n, q0=q0, pv=pv, state=state):
                        def epi():
                            ov = ovp.tile([65, 512], F32)
                            nc.scalar.copy(ov[:], pv[:])
                            ot = miscps.tile([128, 260], F32, tag="m")
                            for j in range(4):
                                nc.tensor.transpose(
                                    ot[:, j * 65:(j + 1) * 65],
                                    ov[:, j * 128:(j + 1) * 128],
                                    ident[:65, :65],
                                )
                            c = csb.tile([128, 4], F32)
                            otv = ot[:].rearrange("p (j c) -> p j c", c=65)
                            nc.vector.reciprocal(c[:], otv[:, :, 64])
                            o_sb = osb.tile([128, 4 * D], F32)
                            for j in range(4):
                                nc.vector.tensor_scalar_mul(
                                    o_sb[:, j * D:(j + 1) * D],
                                    ot[:, j * 65:j * 65 + 64],
                                    c[:, j:j + 1],
                                )
                            nc.sync.dma_start(
                                o_d.ap()[n, q0:q0 + 512, :].rearrange(
                                    "(j p) d -> p j d", p=128
                                ),
                                o_sb[:].rearrange("p (j d) -> p j d", d=D),
                            )
                            state["c"] = c
                        return epi

                    pending.append(mk_epi())

                    def mk_wgroup(n=n, q0=q0, j=0, kg=0, p_tiles=p_tiles,
                                  state=state):
                        def wg():
                            if kg == 0:
                                state["w_row", j] = wsb.tile([128, S], F32, tag="w", name="w_row")
                            w_row = state["w_row", j]
                            wt = miscps.tile([128, 512], F32R, tag="m")
                            for u in range(4):
                                kc = kg * 4 + u
                                nc.tensor.transpose(
                                    wt[:, u * 128:(u + 1) * 128],
                                    p_tiles[kc][:, j * 128:(j + 1) * 128],
                                    ident_r[:, :128],
                                )
                            nc.vector.tensor_scalar_mul(
                                w_row[:, kg * 512:(kg + 1) * 512],
                                wt[:].bitcast(F32),
                                state["c"][:, j:j + 1],
                            )
                            if kg == 3:
                                nc.sync.dma_start(
                                    w_d.ap()[n, q0 + j * 128:q0 + (j + 1) * 128, :],
                                    w_row[:],
                                )
                        return wg

                    for j in range(4):
                        for kg in range(4):
                            pending.append(mk_wgroup(j=j, kg=kg))

            while pending:
                drain()

        if repeat > 1:
            with tc.For_i(0, repeat, 1):
                body()
        else:
            body()

    nc.compile()
    return nc


def host_inputs(query, key, value, padding_mask):
    """Split full inputs into 8 per-core input maps."""
    query = np.ascontiguousarray(query, dtype=np.float32)
    key = np.ascontiguousarray(key, dtype=np.float32)
    value = np.ascontiguousarray(value, dtype=np.float32)
    in_maps = []
    tri = np.arange(128)[None, :] >= np.arange(128)[:, None]  # [p, j]: j >= p
    for c in range(NCORES):
        b = c // CORES_PER_B
        h0 = (c % CORES_PER_B) * HPC
        pad = padding_mask[b, 0, 0, :].astype(np.float32)  # [S]
        a = NEG * pad
        bv = NEG * (1.0 - pad)
        A = a.reshape(NT, 128)
        BV = bv.reshape(NT, 128)
        # diag[p, t*128+j] = (j>=p) ? a[t*128+p] : bv[t*128+p]
        dgt = 8.0 * np.where(tri[None, :, :], A[:, :, None], BV[:, :, None])  # [t,p,j]
        in_maps.append({
            "q": query[b, h0:h0 + HPC],
            "k": key[b, h0:h0 + HPC],
            "v": value[b, h0:h0 + HPC],
            "biasa": np.ascontiguousarray(A.T),
            "biasb": np.ascontiguousarray(BV.T),
            "diag": np.ascontiguousarray(dgt.transpose(1, 0, 2).reshape(128, S)),
        })
    return in_maps


_cache = {}


def _get_program(repeat=1):
    key = (HPC, repeat)
    if key not in _cache:
        _cache[key] = build(HPC, repeat=repeat)
    return _cache[key]


def kernel(query, key, value, padding_mask):
    nc = _get_program(repeat=int(os.environ.get("KERNEL_REPEAT", "1")))
    in_maps = host_inputs(query, key, value, padding_mask)
    res = run_bass_kernel_spmd(nc, in_maps, core_ids=list(range(NCORES)))
    out = np.empty((B, H, S, D), dtype=np.float32)
    wts = np.empty((B, H, S, S), dtype=np.float32)
    for c in range(NCORES):
        b = c // CORES_PER_B
        h0 = (c % CORES_PER_B) * HPC
        out[b, h0:h0 + HPC] = res.results[c]["o"]
        wts[b, h0:h0 + HPC] = res.results[c]["w"]
    return out, wts


# revision 23
# speedup vs baseline: 552.9299x; 552.9299x over previous
"""Causal+padding-mask attention kernel for 8 Trainium2 NeuronCores.

Problem: B=4, H=16, S=2048, D=64 fp32 attention that returns BOTH the
attention output [B,H,S,D] and the normalized attention weights
[B,H,S,S] (the reference's softmax over masked scores).

Reference mask quirk: scores are masked where (future + pad) == 1, i.e.
future XOR pad. A "future" position whose key padding bit is 1 is
UNMASKED (raw score kept).

Sharding: 64 (b,h) pairs over 8 cores -> core c handles batch b=c//2,
heads h = (c%2)*8 .. +8. No inter-core communication.

Per-core kernel design (T-layout primary):
  - Q,K loaded [128q,64d] tiles, PE-transposed to qT,kT [64d, 2048].
  - S_T[k,q] = K @ Q^T computed with k on partitions via
    matmul(lhsT=kT slice, rhs=qT slice) in float32r (full rate at N=512).
  - Masking: in T layout the mask value depends (off-diagonal) only on k
    = the partition index, so it folds into the per-partition bias of the
    ACT exp: p = Exp(0.125*S_T + bias[k]). bias_a[k] = -1e9*pad[k] for
    "past" tiles, bias_b[k] = -1e9*(1-pad[k]) for "future" tiles. Only
    the 16 diagonal 128x128 tiles need an elementwise fix (DVE, tiny).
    No row-max subtraction is needed: scaled scores are ~N(0,1), max ~6,
    exp stays in fp32 range.
  - PV: matmul(lhsT=Vp[kc] [128k,65], rhs=p [128k,512q]) accumulates
    out_T[d,q] AND the softmax denominator (ones column appended to V).
  - Weights out: PE-transpose p tiles back to [q,k], normalize by 1/denom
    (per-partition in q layout) fused into the PSUM->SBUF copy on DVE,
    DMA [128,2048] fp32 rows to HBM.
  - Output out[q,d]: small PE transposes of out_T + per-partition scale.
"""

import os
from contextlib import ExitStack

import numpy as np

import concourse.bacc as bacc
import concourse.mybir as mybir
import concourse.tile as tile
from concourse.bass_utils import run_bass_kernel_spmd
from concourse.masks import make_identity

B, H, S, D = 4, 16, 2048, 64
NCORES = 8
CORES_PER_B = NCORES // B          # 2
HPC = H // CORES_PER_B             # 8 heads per core
NT = S // 128                      # 16 k/q tiles of 128
F32 = mybir.dt.float32
F32R = mybir.dt.float32r
EXP = mybir.ActivationFunctionType.Exp
NEG = -1.0e9


def build(nP=HPC, repeat=1, timing=False):
    """Build the per-core Bass program processing nP (b,h) pairs.

    timing=True keeps all compute/DMA identical but makes the big outputs
    Internal DRAM (not transferred by PJRT) so wall-clock deltas over
    For_i repeats measure on-device execution, not host transfers.
    """
    nc = bacc.Bacc("TRN2", target_bir_lowering=False, debug=False)

    okind = "Internal" if timing else "ExternalOutput"
    q_d = nc.dram_tensor("q", [nP, S, D], F32, kind="ExternalInput")
    k_d = nc.dram_tensor("k", [nP, S, D], F32, kind="ExternalInput")
    v_d = nc.dram_tensor("v", [nP, S, D], F32, kind="ExternalInput")
    ba_d = nc.dram_tensor("biasa", [128, NT], F32, kind="ExternalInput")
    bb_d = nc.dram_tensor("biasb", [128, NT], F32, kind="ExternalInput")
    dg_d = nc.dram_tensor("diag", [128, S], F32, kind="ExternalInput")
    w_d = nc.dram_tensor("w", [nP, S, S], F32, kind=okind)
    o_d = nc.dram_tensor("o", [nP, S, D], F32, kind=okind)
    dum_d = nc.dram_tensor("tdum", [1, 4], F32, kind="ExternalOutput") if timing else None

    with tile.TileContext(nc) as tc, ExitStack() as ctx:
        const = ctx.enter_context(tc.tile_pool(name="const", bufs=1))
        ident = const.tile([128, 128], F32)
        make_identity(nc, ident[:])
        ident_r = const.tile([128, 128], F32R)
        nc.scalar.copy(ident_r[:], ident[:])
        ba = const.tile([128, NT], F32)
        nc.sync.dma_start(ba[:], ba_d.ap())
        bb = const.tile([128, NT], F32)
        nc.sync.dma_start(bb[:], bb_d.ap())
        dg = const.tile([128, S], F32)
        nc.sync.dma_start(dg[:], dg_d.ap())

        natp = ctx.enter_context(tc.tile_pool(name="nat", bufs=2))
        ktp = ctx.enter_context(tc.tile_pool(name="kt", bufs=2))
        vp = ctx.enter_context(tc.tile_pool(name="v", bufs=2))
        pp = ctx.enter_context(tc.tile_pool(name="p", bufs=34))
        wsb = ctx.enter_context(tc.tile_pool(name="wsb", bufs=3))
        ovp = ctx.enter_context(tc.tile_pool(name="ov", bufs=2))
        osb = ctx.enter_context(tc.tile_pool(name="osb", bufs=2))
        csb = ctx.enter_context(tc.tile_pool(name="c", bufs=2))
        dtp = ctx.enter_context(tc.tile_pool(name="dtmp", bufs=2))

        sps = ctx.enter_context(tc.tile_pool(name="sps", bufs=3, space="PSUM"))
        pvps = ctx.enter_context(tc.tile_pool(name="pvps", bufs=1, space="PSUM"))
        miscps = ctx.enter_context(tc.tile_pool(name="mps", bufs=4, space="PSUM"))

        PVLAG = 3

        def body():
            pending = []

            def drain():
                if pending:
                    pending.pop(0)()

            for n in range(nP):
                # ---- load Q,K,V; build qT,kT [64, S] and Vp [128, 16*65]
                q_nat = natp.tile([128, NT * D], F32, tag="qnat")
                k_nat = natp.tile([128, NT * D], F32, tag="knat")
                nc.sync.dma_start(
                    q_nat[:].rearrange("p (t d) -> p t d", d=D),
                    q_d.ap()[n].rearrange("(t p) d -> p t d", p=128),
                )
                nc.sync.dma_start(
                    k_nat[:].rearrange("p (t d) -> p t d", d=D),
                    k_d.ap()[n].rearrange("(t p) d -> p t d", p=128),
                )
                qT = ktp.tile([64, S], F32R, tag="qT")
                kT = ktp.tile([64, S], F32R, tag="kT")
                for src, dst in ((q_nat, qT), (k_nat, kT)):
                    for g in range(4):
                        tp = sps.tile([64, 512], F32, tag="s", name="tp")
                        for u in range(4):
                            t = g * 4 + u
                            nc.tensor.transpose(
                                tp[:, u * 128:(u + 1) * 128],
                                src[:, t * D:(t + 1) * D],
                                ident[:, :128],
                            )
                        nc.scalar.copy(dst[:, g * 512:(g + 1) * 512], tp[:])
                v_sb = vp.tile([128, NT * 65], F32)
                vv = v_sb[:].rearrange("p (t c) -> p t c", c=65)
                nc.gpsimd.memset(vv[:, :, 64], 1.0)
                nc.sync.dma_start(
                    vv[:, :, 0:64],
                    v_d.ap()[n].rearrange("(t p) d -> p t d", p=128),
                )
                v_r = vp.tile([128, NT * 65], F32R, tag="vr")
                nc.scalar.copy(v_r[:], v_sb[:])

                for qc in range(4):
                    q0 = qc * 512
                    pv = pvps.tile([65, 512], F32)
                    p_tiles = []
                    pv_jobs = []
                    for kc in range(NT):
                        s_ps = sps.tile([128, 512], F32, tag="s")
                        nc.tensor.matmul(
                            s_ps[:],
                            lhsT=kT[:, kc * 128:(kc + 1) * 128],
                            rhs=qT[:, q0:q0 + 512],
                            start=True,
                            stop=True,
                        )
                        p_sb = pp.tile([128, 512], F32R, tag="p")
                        jstar = kc - 4 * qc
                        if 0 <= jstar < 4:
                            lo, hi = jstar * 128, jstar * 128 + 128
                            if lo > 0:
                                nc.scalar.activation(
                                    p_sb[:, :lo], s_ps[:, :lo], EXP,
                                    bias=bb[:, kc:kc + 1], scale=0.125,
                                )
                            if hi < 512:
                                nc.scalar.activation(
                                    p_sb[:, hi:], s_ps[:, hi:], EXP,
                                    bias=ba[:, kc:kc + 1], scale=0.125,
                                )
                            tmp = dtp.tile([128, 128], F32, tag="d")
                            nc.vector.tensor_add(
                                tmp[:], s_ps[:, lo:hi], dg[:, kc * 128:(kc + 1) * 128]
                            )
                            nc.scalar.activation(
                                p_sb[:, lo:hi], tmp[:], EXP, scale=0.125
                            )
                        else:
                            bias = ba if kc < 4 * qc else bb
                            nc.scalar.activation(
                                p_sb[:], s_ps[:], EXP,
                                bias=bias[:, kc:kc + 1], scale=0.125,
                            )
                        p_tiles.append(p_sb)

                        def mk_pv(kc=kc, p_sb=p_sb, pv=pv, v_r=v_r):
                            def f():
                                nc.tensor.matmul(
                                    pv[:],
                                    lhsT=v_r[:, kc * 65:(kc + 1) * 65],
                                    rhs=p_sb[:],
                                    start=(kc == 0),
                                    stop=(kc == NT - 1),
                                )
                            return f

                        pv_jobs.append(mk_pv())
                        if kc >= PVLAG:
                            pv_jobs[kc - PVLAG]()
                        drain()
                    for kc in range(NT - PVLAG, NT):
                        pv_jobs[kc]()
                    drain()

                    # defer epilogue + weights phase into the next q-block's
                    # kc loop so PE/ACT/DVE stay continuously fed
                    state = {}

                    def mk_epi(n=n, q0=q0, pv=pv, state=state):
                        def epi():
                            ov = ovp.tile([65, 512], F32)
                            nc.scalar.copy(ov[:], pv[:])
                            ot = miscps.tile([128, 260], F32, tag="m")
                            for j in range(4):
                                nc.tensor.transpose(
                                    ot[:, j * 65:(j + 1) * 65],
                                    ov[:, j * 128:(j + 1) * 128],
                                    ident[:65, :65],
                                )
                            c = csb.tile([128, 4], F32)
                            otv = ot[:].rearrange("p (j c) -> p j c", c=65)
                            nc.vector.reciprocal(c[:], otv[:, :, 64])
                            o_sb = osb.tile([128, 4 * D], F32)
                            for j in range(4):
                                nc.vector.tensor_scalar_mul(
                                    o_sb[:, j * D:(j + 1) * D],
                                    ot[:, j * 65:j * 65 + 64],
                                    c[:, j:j + 1],
                                )
                            nc.sync.dma_start(
                                o_d.ap()[n, q0:q0 + 512, :].rearrange(
                                    "(j p) d -> p j d", p=128
                                ),
                                o_sb[:].rearrange("p (j d) -> p j d", d=D),
                            )
                            state["c"] = c
                        return epi

                    pending.append(mk_epi())

                    def mk_wgroup(n=n, q0=q0, j=0, kg=0, p_tiles=p_tiles,
                                  state=state):
                        def wg():
                            if kg == 0:
                                state["w_row", j] = wsb.tile([128, S], F32, tag="w", name="w_row")
                            w_row = state["w_row", j]
                            wt = miscps.tile([128, 512], F32R, tag="m")
                            for u in range(4):
                                kc = kg * 4 + u
                                nc.tensor.transpose(
                                    wt[:, u * 128:(u + 1) * 128],
                                    p_tiles[kc][:, j * 128:(j + 1) * 128],
                                    ident_r[:, :128],
                                )
                            nc.vector.tensor_scalar_mul(
                                w_row[:, kg * 512:(kg + 1) * 512],
                                wt[:].bitcast(F32),
                                state["c"][:, j:j + 1],
                            )
                            if kg == 3:
                                nc.sync.dma_start(
                                    w_d.ap()[n, q0 + j * 128:q0 + (j + 1) * 128, :],
                                    w_row[:],
                                )
                        return wg

                    for j in range(4):
                        for kg in range(4):
                            pending.append(mk_wgroup(j=j, kg=kg))

            while pending:
                drain()
            if timing:
                dum = csb.tile([128, 4], F32, name="dum")
                nc.gpsimd.memset(dum[:], 1.0)
                nc.sync.dma_start(dum_d.ap()[:, :], dum[0:1, :])

        if repeat > 1:
            with tc.For_i(0, repeat, 1):
                body()
        else:
            body()

    nc.compile()
    return nc


def host_inputs(query, key, value, padding_mask):
    """Split full inputs into 8 per-core input maps."""
    query = np.ascontiguousarray(query, dtype=np.float32)
    key = np.ascontiguousarray(key, dtype=np.float32)
    value = np.ascontiguousarray(value, dtype=np.float32)
    in_maps = []
    tri = np.arange(128)[None, :] >= np.arange(128)[:, None]  # [p, j]: j >= p
    for c in range(NCORES):
        b = c // CORES_PER_B
        h0 = (c % CORES_PER_B) * HPC
        pad = padding_mask[b, 0, 0, :].astype(np.float32)  # [S]
        a = NEG * pad
        bv = NEG * (1.0 - pad)
        A = a.reshape(NT, 128)
        BV = bv.reshape(NT, 128)
        # diag[p, t*128+j] = (j>=p) ? a[t*128+p] : bv[t*128+p]
        dgt = 8.0 * np.where(tri[None, :, :], A[:, :, None], BV[:, :, None])  # [t,p,j]
        in_maps.append({
            "q": query[b, h0:h0 + HPC],
            "k": key[b, h0:h0 + HPC],
            "v": value[b, h0:h0 + HPC],
            "biasa": np.ascontiguousarray(A.T),
            "biasb": np.ascontiguousarray(BV.T),
            "diag": np.ascontiguousarray(dgt.transpose(1, 0, 2).reshape(128, S)),
        })
    return in_maps


_cache = {}


def _get_program(repeat=1):
    key = (HPC, repeat)
    if key not in _cache:
        _cache[key] = build(HPC, repeat=repeat)
    return _cache[key]


def kernel(query, key, value, padding_mask):
    nc = _get_program(repeat=int(os.environ.get("KERNEL_REPEAT", "1")))
    in_maps = host_inputs(query, key, value, padding_mask)
    res = run_bass_kernel_spmd(nc, in_maps, core_ids=list(range(NCORES)))
    out = np.empty((B, H, S, D), dtype=np.float32)
    wts = np.empty((B, H, S, S), dtype=np.float32)
    for c in range(NCORES):
        b = c // CORES_PER_B
        h0 = (c % CORES_PER_B) * HPC
        out[b, h0:h0 + HPC] = res.results[c]["o"]
        wts[b, h0:h0 + HPC] = res.results[c]["w"]
    return out, wts


# revision 24
# speedup vs baseline: 567.6156x; 1.0266x over previous
"""Causal+padding-mask attention kernel for 8 Trainium2 NeuronCores.

Problem: B=4, H=16, S=2048, D=64 fp32 attention that returns BOTH the
attention output [B,H,S,D] and the normalized attention weights
[B,H,S,S] (the reference's softmax over masked scores).

Reference mask quirk: scores are masked where (future + pad) == 1, i.e.
future XOR pad. A "future" position whose key padding bit is 1 is
UNMASKED (raw score kept).

Sharding: 64 (b,h) pairs over 8 cores -> core c handles batch b=c//2,
heads h = (c%2)*8 .. +8. No inter-core communication.

Per-core kernel design (T-layout primary):
  - Q,K loaded [128q,64d] tiles, PE-transposed to qT,kT [64d, 2048].
  - S_T[k,q] = K @ Q^T computed with k on partitions via
    matmul(lhsT=kT slice, rhs=qT slice) in float32r (full rate at N=512).
  - Masking: in T layout the mask value depends (off-diagonal) only on k
    = the partition index, so it folds into the per-partition bias of the
    ACT exp: p = Exp(0.125*S_T + bias[k]). bias_a[k] = -1e9*pad[k] for
    "past" tiles, bias_b[k] = -1e9*(1-pad[k]) for "future" tiles. Only
    the 16 diagonal 128x128 tiles need an elementwise fix (DVE, tiny).
    No row-max subtraction is needed: scaled scores are ~N(0,1), max ~6,
    exp stays in fp32 range.
  - PV: matmul(lhsT=Vp[kc] [128k,65], rhs=p [128k,512q]) accumulates
    out_T[d,q] AND the softmax denominator (ones column appended to V).
  - Weights out: PE-transpose p tiles back to [q,k], normalize by 1/denom
    (per-partition in q layout) fused into the PSUM->SBUF copy on DVE,
    DMA [128,2048] fp32 rows to HBM.
  - Output out[q,d]: small PE transposes of out_T + per-partition scale.
"""

import os
from contextlib import ExitStack

import numpy as np

import concourse.bacc as bacc
import concourse.mybir as mybir
import concourse.tile as tile
from concourse.bass_utils import run_bass_kernel_spmd
from concourse.masks import make_identity

B, H, S, D = 4, 16, 2048, 64
NCORES = 8
CORES_PER_B = NCORES // B          # 2
HPC = H // CORES_PER_B             # 8 heads per core
NT = S // 128                      # 16 k/q tiles of 128
F32 = mybir.dt.float32
F32R = mybir.dt.float32r
EXP = mybir.ActivationFunctionType.Exp
NEG = -1.0e9


def build(nP=HPC, repeat=1, timing=False):
    """Build the per-core Bass program processing nP (b,h) pairs.

    timing=True keeps all compute/DMA identical but makes the big outputs
    Internal DRAM (not transferred by PJRT) so wall-clock deltas over
    For_i repeats measure on-device execution, not host transfers.
    """
    nc = bacc.Bacc("TRN2", target_bir_lowering=False, debug=False)

    okind = "Internal" if timing else "ExternalOutput"
    q_d = nc.dram_tensor("q", [nP, S, D], F32, kind="ExternalInput")
    k_d = nc.dram_tensor("k", [nP, S, D], F32, kind="ExternalInput")
    v_d = nc.dram_tensor("v", [nP, S, D], F32, kind="ExternalInput")
    ba_d = nc.dram_tensor("biasa", [128, NT], F32, kind="ExternalInput")
    bb_d = nc.dram_tensor("biasb", [128, NT], F32, kind="ExternalInput")
    dg_d = nc.dram_tensor("diag", [128, S], F32, kind="ExternalInput")
    w_d = nc.dram_tensor("w", [nP, S, S], F32, kind=okind)
    o_d = nc.dram_tensor("o", [nP, S, D], F32, kind=okind)
    dum_d = nc.dram_tensor("tdum", [1, 4], F32, kind="ExternalOutput") if timing else None

    with tile.TileContext(nc) as tc, ExitStack() as ctx:
        const = ctx.enter_context(tc.tile_pool(name="const", bufs=1))
        ident = const.tile([128, 128], F32)
        make_identity(nc, ident[:])
        ident_r = const.tile([128, 128], F32R)
        nc.scalar.copy(ident_r[:], ident[:])
        ba = const.tile([128, NT], F32)
        nc.sync.dma_start(ba[:], ba_d.ap())
        bb = const.tile([128, NT], F32)
        nc.sync.dma_start(bb[:], bb_d.ap())
        dg = const.tile([128, S], F32)
        nc.sync.dma_start(dg[:], dg_d.ap())

        natp = ctx.enter_context(tc.tile_pool(name="nat", bufs=2))
        ktp = ctx.enter_context(tc.tile_pool(name="kt", bufs=2))
        vp = ctx.enter_context(tc.tile_pool(name="v", bufs=2))
        pp = ctx.enter_context(tc.tile_pool(name="p", bufs=34))
        wsb = ctx.enter_context(tc.tile_pool(name="wsb", bufs=3))
        ovp = ctx.enter_context(tc.tile_pool(name="ov", bufs=2))
        osb = ctx.enter_context(tc.tile_pool(name="osb", bufs=2))
        csb = ctx.enter_context(tc.tile_pool(name="c", bufs=2))
        dtp = ctx.enter_context(tc.tile_pool(name="dtmp", bufs=2))

        sps = ctx.enter_context(tc.tile_pool(name="sps", bufs=3, space="PSUM"))
        pvps = ctx.enter_context(tc.tile_pool(name="pvps", bufs=1, space="PSUM"))
        miscps = ctx.enter_context(tc.tile_pool(name="mps", bufs=4, space="PSUM"))

        PVLAG = 3

        def body():
            pending = []

            def drain():
                if pending:
                    pending.pop(0)()

            for n in range(nP):
                # ---- load Q,K,V; build qT,kT [64, S] and Vp [128, 16*65]
                q_nat = natp.tile([128, NT * D], F32, tag="qnat")
                k_nat = natp.tile([128, NT * D], F32, tag="knat")
                # contiguous loads: partition p holds rows p*16..p*16+15;
                # the transpose-copy below restores natural q/k column order
                # via a strided dest AP (q = 16*c + r).
                nc.sync.dma_start(
                    q_nat[:], q_d.ap()[n].rearrange("(p r) d -> p (r d)", p=128)
                )
                nc.sync.dma_start(
                    k_nat[:], k_d.ap()[n].rearrange("(p r) d -> p (r d)", p=128)
                )
                qT = ktp.tile([64, S], F32R, tag="qT")
                kT = ktp.tile([64, S], F32R, tag="kT")
                for src, dst in ((q_nat, qT), (k_nat, kT)):
                    dstv = dst[:].rearrange("z (c r) -> z r c", r=NT)
                    for g in range(4):
                        tp = sps.tile([64, 512], F32, tag="s", name="tp")
                        for u in range(4):
                            r = g * 4 + u
                            nc.tensor.transpose(
                                tp[:, u * 128:(u + 1) * 128],
                                src[:, r * D:(r + 1) * D],
                                ident[:, :128],
                            )
                        nc.scalar.copy(
                            dstv[:, g * 4:(g + 1) * 4, :],
                            tp[:].rearrange("z (u c) -> z u c", c=128),
                        )
                v_sb = vp.tile([128, NT * 65], F32)
                vv = v_sb[:].rearrange("p (t c) -> p t c", c=65)
                nc.gpsimd.memset(vv[:, :, 64], 1.0)
                nc.sync.dma_start(
                    vv[:, :, 0:64],
                    v_d.ap()[n].rearrange("(t p) d -> p t d", p=128),
                )
                v_r = vp.tile([128, NT * 65], F32R, tag="vr")
                nc.scalar.copy(v_r[:], v_sb[:])

                for qc in range(4):
                    q0 = qc * 512
                    pv = pvps.tile([65, 512], F32)
                    p_tiles = []
                    pv_jobs = []
                    for kc in range(NT):
                        s_ps = sps.tile([128, 512], F32, tag="s")
                        nc.tensor.matmul(
                            s_ps[:],
                            lhsT=kT[:, kc * 128:(kc + 1) * 128],
                            rhs=qT[:, q0:q0 + 512],
                            start=True,
                            stop=True,
                        )
                        p_sb = pp.tile([128, 512], F32R, tag="p")
                        jstar = kc - 4 * qc
                        if 0 <= jstar < 4:
                            lo, hi = jstar * 128, jstar * 128 + 128
                            if lo > 0:
                                nc.scalar.activation(
                                    p_sb[:, :lo], s_ps[:, :lo], EXP,
                                    bias=bb[:, kc:kc + 1], scale=0.125,
                                )
                            if hi < 512:
                                nc.scalar.activation(
                                    p_sb[:, hi:], s_ps[:, hi:], EXP,
                                    bias=ba[:, kc:kc + 1], scale=0.125,
                                )
                            tmp = dtp.tile([128, 128], F32, tag="d")
                            nc.vector.tensor_add(
                                tmp[:], s_ps[:, lo:hi], dg[:, kc * 128:(kc + 1) * 128]
                            )
                            nc.scalar.activation(
                                p_sb[:, lo:hi], tmp[:], EXP, scale=0.125
                            )
                        else:
                            bias = ba if kc < 4 * qc else bb
                            nc.scalar.activation(
                                p_sb[:], s_ps[:], EXP,
                                bias=bias[:, kc:kc + 1], scale=0.125,
                            )
                        p_tiles.append(p_sb)

                        def mk_pv(kc=kc, p_sb=p_sb, pv=pv, v_r=v_r):
                            def f():
                                nc.tensor.matmul(
                                    pv[:],
                                    lhsT=v_r[:, kc * 65:(kc + 1) * 65],
                                    rhs=p_sb[:],
                                    start=(kc == 0),
                                    stop=(kc == NT - 1),
                                )
                            return f

                        pv_jobs.append(mk_pv())
                        if kc >= PVLAG:
                            pv_jobs[kc - PVLAG]()
                        drain()
                    for kc in range(NT - PVLAG, NT):
                        pv_jobs[kc]()
                    drain()

                    # defer epilogue + weights phase into the next q-block's
                    # kc loop so PE/ACT/DVE stay continuously fed
                    state = {}

                    def mk_epi(n=n, q0=q0, pv=pv, state=state):
                        def epi():
                            ov = ovp.tile([65, 512], F32)
                            nc.scalar.copy(ov[:], pv[:])
                            ot = miscps.tile([128, 260], F32, tag="m")
                            for j in range(4):
                                nc.tensor.transpose(
                                    ot[:, j * 65:(j + 1) * 65],
                                    ov[:, j * 128:(j + 1) * 128],
                                    ident[:65, :65],
                                )
                            c = csb.tile([128, 4], F32)
                            otv = ot[:].rearrange("p (j c) -> p j c", c=65)
                            nc.vector.reciprocal(c[:], otv[:, :, 64])
                            o_sb = osb.tile([128, 4 * D], F32)
                            for j in range(4):
                                nc.vector.tensor_scalar_mul(
                                    o_sb[:, j * D:(j + 1) * D],
                                    ot[:, j * 65:j * 65 + 64],
                                    c[:, j:j + 1],
                                )
                            nc.sync.dma_start(
                                o_d.ap()[n, q0:q0 + 512, :].rearrange(
                                    "(j p) d -> p j d", p=128
                                ),
                                o_sb[:].rearrange("p (j d) -> p j d", d=D),
                            )
                            state["c"] = c
                        return epi

                    pending.append(mk_epi())

                    def mk_wgroup(n=n, q0=q0, j=0, kg=0, p_tiles=p_tiles,
                                  state=state):
                        def wg():
                            if kg == 0:
                                state["w_row", j] = wsb.tile([128, S], F32, tag="w", name="w_row")
                            w_row = state["w_row", j]
                            wt = miscps.tile([128, 512], F32R, tag="m")
                            for u in range(4):
                                kc = kg * 4 + u
                                nc.tensor.transpose(
                                    wt[:, u * 128:(u + 1) * 128],
                                    p_tiles[kc][:, j * 128:(j + 1) * 128],
                                    ident_r[:, :128],
                                )
                            nc.vector.tensor_scalar_mul(
                                w_row[:, kg * 512:(kg + 1) * 512],
                                wt[:].bitcast(F32),
                                state["c"][:, j:j + 1],
                            )
                            if kg == 3:
                                nc.sync.dma_start(
                                    w_d.ap()[n, q0 + j * 128:q0 + (j + 1) * 128, :],
                                    w_row[:],
                                )
                        return wg

                    for j in range(4):
                        for kg in range(4):
                            pending.append(mk_wgroup(j=j, kg=kg))

            while pending:
                drain()
            if timing:
                dum = csb.tile([128, 4], F32, name="dum")
                nc.gpsimd.memset(dum[:], 1.0)
                nc.sync.dma_start(dum_d.ap()[:, :], dum[0:1, :])

        if repeat > 1:
            with tc.For_i(0, repeat, 1):
                body()
        else:
            body()

    nc.compile()
    return nc


def host_inputs(query, key, value, padding_mask):
    """Split full inputs into 8 per-core input maps."""
    query = np.ascontiguousarray(query, dtype=np.float32)
    key = np.ascontiguousarray(key, dtype=np.float32)
    value = np.ascontiguousarray(value, dtype=np.float32)
    in_maps = []
    tri = np.arange(128)[None, :] >= np.arange(128)[:, None]  # [p, j]: j >= p
    for c in range(NCORES):
        b = c // CORES_PER_B
        h0 = (c % CORES_PER_B) * HPC
        pad = padding_mask[b, 0, 0, :].astype(np.float32)  # [S]
        a = NEG * pad
        bv = NEG * (1.0 - pad)
        A = a.reshape(NT, 128)
        BV = bv.reshape(NT, 128)
        # diag[p, t*128+j] = (j>=p) ? a[t*128+p] : bv[t*128+p]
        dgt = 8.0 * np.where(tri[None, :, :], A[:, :, None], BV[:, :, None])  # [t,p,j]
        in_maps.append({
            "q": query[b, h0:h0 + HPC],
            "k": key[b, h0:h0 + HPC],
            "v": value[b, h0:h0 + HPC],
            "biasa": np.ascontiguousarray(A.T),
            "biasb": np.ascontiguousarray(BV.T),
            "diag": np.ascontiguousarray(dgt.transpose(1, 0, 2).reshape(128, S)),
        })
    return in_maps


_cache = {}


def _get_program(repeat=1):
    key = (HPC, repeat)
    if key not in _cache:
        _cache[key] = build(HPC, repeat=repeat)
    return _cache[key]


def kernel(query, key, value, padding_mask):
    nc = _get_program(repeat=int(os.environ.get("KERNEL_REPEAT", "1")))
    in_maps = host_inputs(query, key, value, padding_mask)
    res = run_bass_kernel_spmd(nc, in_maps, core_ids=list(range(NCORES)))
    out = np.empty((B, H, S, D), dtype=np.float32)
    wts = np.empty((B, H, S, S), dtype=np.float32)
    for c in range(NCORES):
        b = c // CORES_PER_B
        h0 = (c % CORES_PER_B) * HPC
        out[b, h0:h0 + HPC] = res.results[c]["o"]
        wts[b, h0:h0 + HPC] = res.results[c]["w"]
    return out, wts


# revision 27
# speedup vs baseline: 572.5570x; 1.0087x over previous
"""Causal+padding-mask attention kernel for 8 Trainium2 NeuronCores.

Problem: B=4, H=16, S=2048, D=64 fp32 attention that returns BOTH the
attention output [B,H,S,D] and the normalized attention weights
[B,H,S,S] (the reference's softmax over masked scores).

Reference mask quirk: scores are masked where (future + pad) == 1, i.e.
future XOR pad. A "future" position whose key padding bit is 1 is
UNMASKED (raw score kept).

Sharding: 64 (b,h) pairs over 8 cores -> core c handles batch b=c//2,
heads h = (c%2)*8 .. +8. No inter-core communication.

Per-core kernel design (T-layout primary):
  - Q,K loaded [128q,64d] tiles, PE-transposed to qT,kT [64d, 2048].
  - S_T[k,q] = K @ Q^T computed with k on partitions via
    matmul(lhsT=kT slice, rhs=qT slice) in float32r (full rate at N=512).
  - Masking: in T layout the mask value depends (off-diagonal) only on k
    = the partition index, so it folds into the per-partition bias of the
    ACT exp: p = Exp(0.125*S_T + bias[k]). bias_a[k] = -1e9*pad[k] for
    "past" tiles, bias_b[k] = -1e9*(1-pad[k]) for "future" tiles. Only
    the 16 diagonal 128x128 tiles need an elementwise fix (DVE, tiny).
    No row-max subtraction is needed: scaled scores are ~N(0,1), max ~6,
    exp stays in fp32 range.
  - PV: matmul(lhsT=Vp[kc] [128k,65], rhs=p [128k,512q]) accumulates
    out_T[d,q] AND the softmax denominator (ones column appended to V).
  - Weights out: PE-transpose p tiles back to [q,k], normalize by 1/denom
    (per-partition in q layout) fused into the PSUM->SBUF copy on DVE,
    DMA [128,2048] fp32 rows to HBM.
  - Output out[q,d]: small PE transposes of out_T + per-partition scale.
"""

import os
from contextlib import ExitStack

import numpy as np

import concourse.bacc as bacc
import concourse.mybir as mybir
import concourse.tile as tile
from concourse.bass_utils import run_bass_kernel_spmd
from concourse.masks import make_identity

B, H, S, D = 4, 16, 2048, 64
NCORES = 8
CORES_PER_B = NCORES // B          # 2
HPC = H // CORES_PER_B             # 8 heads per core
NT = S // 128                      # 16 k/q tiles of 128
F32 = mybir.dt.float32
F32R = mybir.dt.float32r
EXP = mybir.ActivationFunctionType.Exp
NEG = -1.0e9


def build(nP=HPC, repeat=1, timing=False):
    """Build the per-core Bass program processing nP (b,h) pairs.

    timing=True keeps all compute/DMA identical but makes the big outputs
    Internal DRAM (not transferred by PJRT) so wall-clock deltas over
    For_i repeats measure on-device execution, not host transfers.
    """
    nc = bacc.Bacc("TRN2", target_bir_lowering=False, debug=False)

    okind = "Internal" if timing else "ExternalOutput"
    q_d = nc.dram_tensor("q", [nP, S, D], F32, kind="ExternalInput")
    k_d = nc.dram_tensor("k", [nP, S, D], F32, kind="ExternalInput")
    v_d = nc.dram_tensor("v", [nP, S, D], F32, kind="ExternalInput")
    ba_d = nc.dram_tensor("biasa", [128, NT], F32, kind="ExternalInput")
    bb_d = nc.dram_tensor("biasb", [128, NT], F32, kind="ExternalInput")
    dg_d = nc.dram_tensor("diag", [128, S], F32, kind="ExternalInput")
    w_d = nc.dram_tensor("w", [nP, S, S], F32, kind=okind)
    o_d = nc.dram_tensor("o", [nP, S, D], F32, kind=okind)
    dum_d = nc.dram_tensor("tdum", [1, 4], F32, kind="ExternalOutput") if timing else None

    with tile.TileContext(nc) as tc, ExitStack() as ctx:
        const = ctx.enter_context(tc.tile_pool(name="const", bufs=1))
        ident = const.tile([128, 128], F32)
        make_identity(nc, ident[:])
        ident_r = const.tile([128, 128], F32R)
        nc.scalar.copy(ident_r[:], ident[:])
        ba = const.tile([128, NT], F32)
        nc.sync.dma_start(ba[:], ba_d.ap())
        bb = const.tile([128, NT], F32)
        nc.sync.dma_start(bb[:], bb_d.ap())
        dg = const.tile([128, S], F32)
        nc.sync.dma_start(dg[:], dg_d.ap())

        natp = ctx.enter_context(tc.tile_pool(name="nat", bufs=2))
        ktp = ctx.enter_context(tc.tile_pool(name="kt", bufs=2))
        vp = ctx.enter_context(tc.tile_pool(name="v", bufs=2))
        pp = ctx.enter_context(tc.tile_pool(name="p", bufs=34))
        wsb = ctx.enter_context(tc.tile_pool(name="wsb", bufs=3))
        ovp = ctx.enter_context(tc.tile_pool(name="ov", bufs=2))
        osb = ctx.enter_context(tc.tile_pool(name="osb", bufs=2))
        csb = ctx.enter_context(tc.tile_pool(name="c", bufs=2))
        dtp = ctx.enter_context(tc.tile_pool(name="dtmp", bufs=2))

        sps = ctx.enter_context(tc.tile_pool(name="sps", bufs=3, space="PSUM"))
        pvps = ctx.enter_context(tc.tile_pool(name="pvps", bufs=1, space="PSUM"))
        miscps = ctx.enter_context(tc.tile_pool(name="mps", bufs=4, space="PSUM"))

        PVLAG = 3

        def body():
            pending = []

            def drain():
                if pending:
                    pending.pop(0)()

            loaded = {}

            def load_pair(n):
                # contiguous loads: partition p holds rows p*16..p*16+15;
                # the transpose-copy in the prep phase restores natural q/k
                # column order via a strided dest AP (q = 16*c + r).
                q_nat = natp.tile([128, NT * D], F32, tag="qnat", name="q_nat")
                k_nat = natp.tile([128, NT * D], F32, tag="knat", name="k_nat")
                nc.sync.dma_start(
                    q_nat[:], q_d.ap()[n].rearrange("(p r) d -> p (r d)", p=128)
                )
                nc.sync.dma_start(
                    k_nat[:], k_d.ap()[n].rearrange("(p r) d -> p (r d)", p=128)
                )
                v_sb = vp.tile([128, NT * 65], F32, tag="v_sb", name="v_sb")
                vv = v_sb[:].rearrange("p (t c) -> p t c", c=65)
                nc.gpsimd.memset(vv[:, :, 64], 1.0)
                nc.sync.dma_start(
                    vv[:, :, 0:64],
                    v_d.ap()[n].rearrange("(t p) d -> p t d", p=128),
                )
                loaded[n] = (q_nat, k_nat, v_sb)

            load_pair(0)
            for n in range(nP):
                # ---- build qT,kT [64, S] and rounded Vp [128, 16*65]
                q_nat, k_nat, v_sb = loaded.pop(n)
                qT = ktp.tile([64, S], F32R, tag="qT")
                kT = ktp.tile([64, S], F32R, tag="kT")
                for src, dst in ((q_nat, qT), (k_nat, kT)):
                    dstv = dst[:].rearrange("z (c r) -> z r c", r=NT)
                    for g in range(4):
                        tp = sps.tile([64, 512], F32, tag="s", name="tp")
                        for u in range(4):
                            r = g * 4 + u
                            nc.tensor.transpose(
                                tp[:, u * 128:(u + 1) * 128],
                                src[:, r * D:(r + 1) * D],
                                ident[:, :128],
                            )
                        nc.scalar.copy(
                            dstv[:, g * 4:(g + 1) * 4, :],
                            tp[:].rearrange("z (u c) -> z u c", c=128),
                        )
                v_r = vp.tile([128, NT * 65], F32R, tag="vr")
                nc.scalar.copy(v_r[:], v_sb[:])

                for qc in range(4):
                    q0 = qc * 512
                    pv = pvps.tile([65, 512], F32)
                    p_tiles = []
                    pv_jobs = []
                    for kc in range(NT):
                        s_ps = sps.tile([128, 512], F32, tag="s")
                        nc.tensor.matmul(
                            s_ps[:],
                            lhsT=kT[:, kc * 128:(kc + 1) * 128],
                            rhs=qT[:, q0:q0 + 512],
                            start=True,
                            stop=True,
                        )
                        p_sb = pp.tile([128, 512], F32R, tag="p")
                        jstar = kc - 4 * qc
                        if 0 <= jstar < 4:
                            lo, hi = jstar * 128, jstar * 128 + 128
                            if lo > 0:
                                nc.scalar.activation(
                                    p_sb[:, :lo], s_ps[:, :lo], EXP,
                                    bias=bb[:, kc:kc + 1], scale=0.125,
                                )
                            if hi < 512:
                                nc.scalar.activation(
                                    p_sb[:, hi:], s_ps[:, hi:], EXP,
                                    bias=ba[:, kc:kc + 1], scale=0.125,
                                )
                            tmp = dtp.tile([128, 128], F32, tag="d")
                            nc.vector.tensor_add(
                                tmp[:], s_ps[:, lo:hi], dg[:, kc * 128:(kc + 1) * 128]
                            )
                            nc.scalar.activation(
                                p_sb[:, lo:hi], tmp[:], EXP, scale=0.125
                            )
                        else:
                            bias = ba if kc < 4 * qc else bb
                            nc.scalar.activation(
                                p_sb[:], s_ps[:], EXP,
                                bias=bias[:, kc:kc + 1], scale=0.125,
                            )
                        p_tiles.append(p_sb)

                        def mk_pv(kc=kc, p_sb=p_sb, pv=pv, v_r=v_r):
                            def f():
                                nc.tensor.matmul(
                                    pv[:],
                                    lhsT=v_r[:, kc * 65:(kc + 1) * 65],
                                    rhs=p_sb[:],
                                    start=(kc == 0),
                                    stop=(kc == NT - 1),
                                )
                            return f

                        pv_jobs.append(mk_pv())
                        if kc >= PVLAG:
                            pv_jobs[kc - PVLAG]()
                        drain()
                    for kc in range(NT - PVLAG, NT):
                        pv_jobs[kc]()
                    drain()
                    if qc == 0 and n + 1 < nP:
                        load_pair(n + 1)

                    # defer epilogue + weights phase into the next q-block's
                    # kc loop so PE/ACT/DVE stay continuously fed
                    state = {}

                    def mk_epi(n=n, q0=q0, pv=pv, state=state):
                        def epi():
                            ov = ovp.tile([65, 512], F32)
                            nc.scalar.copy(ov[:], pv[:])
                            ot = miscps.tile([128, 260], F32, tag="m")
                            for j in range(4):
                                nc.tensor.transpose(
                                    ot[:, j * 65:(j + 1) * 65],
                                    ov[:, j * 128:(j + 1) * 128],
                                    ident[:65, :65],
                                )
                            c = csb.tile([128, 4], F32)
                            otv = ot[:].rearrange("p (j c) -> p j c", c=65)
                            nc.vector.reciprocal(c[:], otv[:, :, 64])
                            o_sb = osb.tile([128, 4 * D], F32)
                            for j in range(4):
                                nc.vector.tensor_scalar_mul(
                                    o_sb[:, j * D:(j + 1) * D],
                                    ot[:, j * 65:j * 65 + 64],
                                    c[:, j:j + 1],
                                )
                            nc.sync.dma_start(
                                o_d.ap()[n, q0:q0 + 512, :].rearrange(
                                    "(j p) d -> p j d", p=128
                                ),
                                o_sb[:].rearrange("p (j d) -> p j d", d=D),
                            )
                            state["c"] = c
                        return epi

                    pending.append(mk_epi())

                    def mk_wgroup(n=n, q0=q0, j=0, kg=0, p_tiles=p_tiles,
                                  state=state):
                        def wg():
                            if kg == 0:
                                state["w_row", j] = wsb.tile([128, S], F32, tag="w", name="w_row")
                            w_row = state["w_row", j]
                            wt = miscps.tile([128, 512], F32R, tag="m")
                            for u in range(4):
                                kc = kg * 4 + u
                                nc.tensor.transpose(
                                    wt[:, u * 128:(u + 1) * 128],
                                    p_tiles[kc][:, j * 128:(j + 1) * 128],
                                    ident_r[:, :128],
                                )
                            nc.vector.tensor_scalar_mul(
                                w_row[:, kg * 512:(kg + 1) * 512],
                                wt[:].bitcast(F32),
                                state["c"][:, j:j + 1],
                            )
                            if kg == 3:
                                nc.sync.dma_start(
                                    w_d.ap()[n, q0 + j * 128:q0 + (j + 1) * 128, :],
                                    w_row[:],
                                )
                        return wg

                    for j in range(4):
                        for kg in range(4):
                            pending.append(mk_wgroup(j=j, kg=kg))

            while pending:
                drain()
            if timing:
                dum = csb.tile([128, 4], F32, name="dum")
                nc.gpsimd.memset(dum[:], 1.0)
                nc.sync.dma_start(dum_d.ap()[:, :], dum[0:1, :])

        if repeat > 1:
            with tc.For_i(0, repeat, 1):
                body()
        else:
            body()

    nc.compile()
    return nc


def host_inputs(query, key, value, padding_mask):
    """Split full inputs into 8 per-core input maps."""
    query = np.ascontiguousarray(query, dtype=np.float32)
    key = np.ascontiguousarray(key, dtype=np.float32)
    value = np.ascontiguousarray(value, dtype=np.float32)
    in_maps = []
    tri = np.arange(128)[None, :] >= np.arange(128)[:, None]  # [p, j]: j >= p
    for c in range(NCORES):
        b = c // CORES_PER_B
        h0 = (c % CORES_PER_B) * HPC
        pad = padding_mask[b, 0, 0, :].astype(np.float32)  # [S]
        a = NEG * pad
        bv = NEG * (1.0 - pad)
        A = a.reshape(NT, 128)
        BV = bv.reshape(NT, 128)
        # diag[p, t*128+j] = (j>=p) ? a[t*128+p] : bv[t*128+p]
        dgt = 8.0 * np.where(tri[None, :, :], A[:, :, None], BV[:, :, None])  # [t,p,j]
        in_maps.append({
            "q": query[b, h0:h0 + HPC],
            "k": key[b, h0:h0 + HPC],
            "v": value[b, h0:h0 + HPC],
            "biasa": np.ascontiguousarray(A.T),
            "biasb": np.ascontiguousarray(BV.T),
            "diag": np.ascontiguousarray(dgt.transpose(1, 0, 2).reshape(128, S)),
        })
    return in_maps


_cache = {}


def _get_program(repeat=1):
    key = (HPC, repeat)
    if key not in _cache:
        _cache[key] = build(HPC, repeat=repeat)
    return _cache[key]


def kernel(query, key, value, padding_mask):
    nc = _get_program(repeat=int(os.environ.get("KERNEL_REPEAT", "1")))
    in_maps = host_inputs(query, key, value, padding_mask)
    res = run_bass_kernel_spmd(nc, in_maps, core_ids=list(range(NCORES)))
    out = np.empty((B, H, S, D), dtype=np.float32)
    wts = np.empty((B, H, S, S), dtype=np.float32)
    for c in range(NCORES):
        b = c // CORES_PER_B
        h0 = (c % CORES_PER_B) * HPC
        out[b, h0:h0 + HPC] = res.results[c]["o"]
        wts[b, h0:h0 + HPC] = res.results[c]["w"]
    return out, wts


# revision 28
# speedup vs baseline: 577.4522x; 1.0085x over previous
"""Causal+padding-mask attention kernel for 8 Trainium2 NeuronCores.

Problem: B=4, H=16, S=2048, D=64 fp32 attention that returns BOTH the
attention output [B,H,S,D] and the normalized attention weights
[B,H,S,S] (the reference's softmax over masked scores).

Reference mask quirk: scores are masked where (future + pad) == 1, i.e.
future XOR pad. A "future" position whose key padding bit is 1 is
UNMASKED (raw score kept).

Sharding: 64 (b,h) pairs over 8 cores -> core c handles batch b=c//2,
heads h = (c%2)*8 .. +8. No inter-core communication.

Per-core kernel design (T-layout primary):
  - Q,K loaded [128q,64d] tiles, PE-transposed to qT,kT [64d, 2048].
  - S_T[k,q] = K @ Q^T computed with k on partitions via
    matmul(lhsT=kT slice, rhs=qT slice) in float32r (full rate at N=512).
  - Masking: in T layout the mask value depends (off-diagonal) only on k
    = the partition index, so it folds into the per-partition bias of the
    ACT exp: p = Exp(0.125*S_T + bias[k]). bias_a[k] = -1e9*pad[k] for
    "past" tiles, bias_b[k] = -1e9*(1-pad[k]) for "future" tiles. Only
    the 16 diagonal 128x128 tiles need an elementwise fix (DVE, tiny).
    No row-max subtraction is needed: scaled scores are ~N(0,1), max ~6,
    exp stays in fp32 range.
  - PV: matmul(lhsT=Vp[kc] [128k,65], rhs=p [128k,512q]) accumulates
    out_T[d,q] AND the softmax denominator (ones column appended to V).
  - Weights out: PE-transpose p tiles back to [q,k], normalize by 1/denom
    (per-partition in q layout) fused into the PSUM->SBUF copy on DVE,
    DMA [128,2048] fp32 rows to HBM.
  - Output out[q,d]: small PE transposes of out_T + per-partition scale.
"""

import os
from contextlib import ExitStack

import numpy as np

import concourse.bacc as bacc
import concourse.mybir as mybir
import concourse.tile as tile
from concourse.bass_utils import run_bass_kernel_spmd
from concourse.masks import make_identity

B, H, S, D = 4, 16, 2048, 64
NCORES = 8
CORES_PER_B = NCORES // B          # 2
HPC = H // CORES_PER_B             # 8 heads per core
NT = S // 128                      # 16 k/q tiles of 128
F32 = mybir.dt.float32
F32R = mybir.dt.float32r
EXP = mybir.ActivationFunctionType.Exp
NEG = -1.0e9


def build(nP=HPC, repeat=1, timing=False):
    """Build the per-core Bass program processing nP (b,h) pairs.

    timing=True keeps all compute/DMA identical but makes the big outputs
    Internal DRAM (not transferred by PJRT) so wall-clock deltas over
    For_i repeats measure on-device execution, not host transfers.
    """
    nc = bacc.Bacc("TRN2", target_bir_lowering=False, debug=False)

    okind = "Internal" if timing else "ExternalOutput"
    q_d = nc.dram_tensor("q", [nP, S, D], F32, kind="ExternalInput")
    k_d = nc.dram_tensor("k", [nP, S, D], F32, kind="ExternalInput")
    v_d = nc.dram_tensor("v", [nP, S, D], F32, kind="ExternalInput")
    ba_d = nc.dram_tensor("biasa", [128, NT], F32, kind="ExternalInput")
    bb_d = nc.dram_tensor("biasb", [128, NT], F32, kind="ExternalInput")
    dg_d = nc.dram_tensor("diag", [128, S], F32, kind="ExternalInput")
    w_d = nc.dram_tensor("w", [nP, S, S], F32, kind=okind)
    o_d = nc.dram_tensor("o", [nP, S, D], F32, kind=okind)
    dum_d = nc.dram_tensor("tdum", [1, 4], F32, kind="ExternalOutput") if timing else None

    with tile.TileContext(nc) as tc, ExitStack() as ctx:
        const = ctx.enter_context(tc.tile_pool(name="const", bufs=1))
        ident = const.tile([128, 128], F32)
        make_identity(nc, ident[:])
        ident_r = const.tile([128, 128], F32R)
        nc.scalar.copy(ident_r[:], ident[:])
        ba = const.tile([128, NT], F32)
        nc.sync.dma_start(ba[:], ba_d.ap())
        bb = const.tile([128, NT], F32)
        nc.sync.dma_start(bb[:], bb_d.ap())
        dg = const.tile([128, S], F32)
        nc.sync.dma_start(dg[:], dg_d.ap())

        natp = ctx.enter_context(tc.tile_pool(name="nat", bufs=3))
        ktp = ctx.enter_context(tc.tile_pool(name="kt", bufs=2))
        vp = ctx.enter_context(tc.tile_pool(name="v", bufs=2))
        pp = ctx.enter_context(tc.tile_pool(name="p", bufs=34))
        wsb = ctx.enter_context(tc.tile_pool(name="wsb", bufs=3))
        ovp = ctx.enter_context(tc.tile_pool(name="ov", bufs=2))
        osb = ctx.enter_context(tc.tile_pool(name="osb", bufs=2))
        csb = ctx.enter_context(tc.tile_pool(name="c", bufs=2))
        dtp = ctx.enter_context(tc.tile_pool(name="dtmp", bufs=2))

        sps = ctx.enter_context(tc.tile_pool(name="sps", bufs=3, space="PSUM"))
        pvps = ctx.enter_context(tc.tile_pool(name="pvps", bufs=1, space="PSUM"))
        miscps = ctx.enter_context(tc.tile_pool(name="mps", bufs=4, space="PSUM"))

        PVLAG = 3

        def body():
            pending = []

            def drain():
                if pending:
                    pending.pop(0)()

            loaded = {}

            def load_pair(n):
                # contiguous loads: partition p holds rows p*16..p*16+15;
                # the transpose-copy in the prep phase restores natural q/k
                # column order via a strided dest AP (q = 16*c + r).
                q_nat = natp.tile([128, NT * D], F32, tag="qnat", name="q_nat")
                k_nat = natp.tile([128, NT * D], F32, tag="knat", name="k_nat")
                nc.sync.dma_start(
                    q_nat[:], q_d.ap()[n].rearrange("(p r) d -> p (r d)", p=128)
                )
                nc.sync.dma_start(
                    k_nat[:], k_d.ap()[n].rearrange("(p r) d -> p (r d)", p=128)
                )
                v_sb = vp.tile([128, NT * 65], F32, tag="v_sb", name="v_sb", bufs=3)
                vv = v_sb[:].rearrange("p (t c) -> p t c", c=65)
                nc.gpsimd.memset(vv[:, :, 64], 1.0)
                nc.sync.dma_start(
                    vv[:, :, 0:64],
                    v_d.ap()[n].rearrange("(t p) d -> p t d", p=128),
                )
                loaded[n] = (q_nat, k_nat, v_sb)

            load_pair(0)
            nl = [1]
            for n in range(nP):
                # ---- build qT,kT [64, S] and rounded Vp [128, 16*65]
                q_nat, k_nat, v_sb = loaded.pop(n)
                qT = ktp.tile([64, S], F32R, tag="qT")
                kT = ktp.tile([64, S], F32R, tag="kT")
                for src, dst in ((q_nat, qT), (k_nat, kT)):
                    dstv = dst[:].rearrange("z (c r) -> z r c", r=NT)
                    for g in range(4):
                        tp = sps.tile([64, 512], F32, tag="s", name="tp")
                        for u in range(4):
                            r = g * 4 + u
                            nc.tensor.transpose(
                                tp[:, u * 128:(u + 1) * 128],
                                src[:, r * D:(r + 1) * D],
                                ident[:, :128],
                            )
                        nc.scalar.copy(
                            dstv[:, g * 4:(g + 1) * 4, :],
                            tp[:].rearrange("z (u c) -> z u c", c=128),
                        )
                v_r = vp.tile([128, NT * 65], F32R, tag="vr")
                nc.scalar.copy(v_r[:], v_sb[:])

                for qc in range(4):
                    q0 = qc * 512
                    pv = pvps.tile([65, 512], F32)
                    p_tiles = []
                    pv_jobs = []
                    for kc in range(NT):
                        s_ps = sps.tile([128, 512], F32, tag="s")
                        nc.tensor.matmul(
                            s_ps[:],
                            lhsT=kT[:, kc * 128:(kc + 1) * 128],
                            rhs=qT[:, q0:q0 + 512],
                            start=True,
                            stop=True,
                        )
                        p_sb = pp.tile([128, 512], F32R, tag="p")
                        jstar = kc - 4 * qc
                        if 0 <= jstar < 4:
                            lo, hi = jstar * 128, jstar * 128 + 128
                            if lo > 0:
                                nc.scalar.activation(
                                    p_sb[:, :lo], s_ps[:, :lo], EXP,
                                    bias=bb[:, kc:kc + 1], scale=0.125,
                                )
                            if hi < 512:
                                nc.scalar.activation(
                                    p_sb[:, hi:], s_ps[:, hi:], EXP,
                                    bias=ba[:, kc:kc + 1], scale=0.125,
                                )
                            tmp = dtp.tile([128, 128], F32, tag="d")
                            nc.vector.tensor_add(
                                tmp[:], s_ps[:, lo:hi], dg[:, kc * 128:(kc + 1) * 128]
                            )
                            nc.scalar.activation(
                                p_sb[:, lo:hi], tmp[:], EXP, scale=0.125
                            )
                        else:
                            bias = ba if kc < 4 * qc else bb
                            nc.scalar.activation(
                                p_sb[:], s_ps[:], EXP,
                                bias=bias[:, kc:kc + 1], scale=0.125,
                            )
                        p_tiles.append(p_sb)

                        def mk_pv(kc=kc, p_sb=p_sb, pv=pv, v_r=v_r):
                            def f():
                                nc.tensor.matmul(
                                    pv[:],
                                    lhsT=v_r[:, kc * 65:(kc + 1) * 65],
                                    rhs=p_sb[:],
                                    start=(kc == 0),
                                    stop=(kc == NT - 1),
                                )
                            return f

                        pv_jobs.append(mk_pv())
                        if kc >= PVLAG:
                            pv_jobs[kc - PVLAG]()
                        drain()
                    for kc in range(NT - PVLAG, NT):
                        pv_jobs[kc]()
                    drain()
                    if qc <= 1 and nl[0] < nP and nl[0] <= n + 2:
                        load_pair(nl[0])
                        nl[0] += 1

                    # defer epilogue + weights phase into the next q-block's
                    # kc loop so PE/ACT/DVE stay continuously fed
                    state = {}

                    def mk_epi(n=n, q0=q0, pv=pv, state=state):
                        def epi():
                            ov = ovp.tile([65, 512], F32)
                            nc.scalar.copy(ov[:], pv[:])
                            ot = miscps.tile([128, 260], F32, tag="m")
                            for j in range(4):
                                nc.tensor.transpose(
                                    ot[:, j * 65:(j + 1) * 65],
                                    ov[:, j * 128:(j + 1) * 128],
                                    ident[:65, :65],
                                )
                            c = csb.tile([128, 4], F32)
                            otv = ot[:].rearrange("p (j c) -> p j c", c=65)
                            nc.vector.reciprocal(c[:], otv[:, :, 64])
                            o_sb = osb.tile([128, 4 * D], F32)
                            for j in range(4):
                                nc.vector.tensor_scalar_mul(
                                    o_sb[:, j * D:(j + 1) * D],
                                    ot[:, j * 65:j * 65 + 64],
                                    c[:, j:j + 1],
                                )
                            nc.sync.dma_start(
                                o_d.ap()[n, q0:q0 + 512, :].rearrange(
                                    "(j p) d -> p j d", p=128
                                ),
                                o_sb[:].rearrange("p (j d) -> p j d", d=D),
                            )
                            state["c"] = c
                        return epi

                    pending.append(mk_epi())

                    def mk_wgroup(n=n, q0=q0, j=0, kg=0, p_tiles=p_tiles,
                                  state=state):
                        def wg():
                            if kg == 0:
                                state["w_row", j] = wsb.tile([128, S], F32, tag="w", name="w_row")
                            w_row = state["w_row", j]
                            wt = miscps.tile([128, 512], F32R, tag="m")
                            for u in range(4):
                                kc = kg * 4 + u
                                nc.tensor.transpose(
                                    wt[:, u * 128:(u + 1) * 128],
                                    p_tiles[kc][:, j * 128:(j + 1) * 128],
                                    ident_r[:, :128],
                                )
                            nc.vector.tensor_scalar_mul(
                                w_row[:, kg * 512:(kg + 1) * 512],
                                wt[:].bitcast(F32),
                                state["c"][:, j:j + 1],
                            )
                            if kg == 3:
                                nc.sync.dma_start(
                                    w_d.ap()[n, q0 + j * 128:q0 + (j + 1) * 128, :],
                                    w_row[:],
                                )
                        return wg

                    for j in range(4):
                        for kg in range(4):
                            pending.append(mk_wgroup(j=j, kg=kg))

            while pending:
                drain()
            if timing:
                dum = csb.tile([128, 4], F32, name="dum")
                nc.gpsimd.memset(dum[:], 1.0)
                nc.sync.dma_start(dum_d.ap()[:, :], dum[0:1, :])

        if repeat > 1:
            with tc.For_i(0, repeat, 1):
                body()
        else:
            body()

    nc.compile()
    return nc


def host_inputs(query, key, value, padding_mask):
    """Split full inputs into 8 per-core input maps."""
    query = np.ascontiguousarray(query, dtype=np.float32)
    key = np.ascontiguousarray(key, dtype=np.float32)
    value = np.ascontiguousarray(value, dtype=np.float32)
    in_maps = []
    tri = np.arange(128)[None, :] >= np.arange(128)[:, None]  # [p, j]: j >= p
    for c in range(NCORES):
        b = c // CORES_PER_B
        h0 = (c % CORES_PER_B) * HPC
        pad = padding_mask[b, 0, 0, :].astype(np.float32)  # [S]
        a = NEG * pad
        bv = NEG * (1.0 - pad)
        A = a.reshape(NT, 128)
        BV = bv.reshape(NT, 128)
        # diag[p, t*128+j] = (j>=p) ? a[t*128+p] : bv[t*128+p]
        dgt = 8.0 * np.where(tri[None, :, :], A[:, :, None], BV[:, :, None])  # [t,p,j]
        in_maps.append({
            "q": query[b, h0:h0 + HPC],
            "k": key[b, h0:h0 + HPC],
            "v": value[b, h0:h0 + HPC],
            "biasa": np.ascontiguousarray(A.T),
            "biasb": np.ascontiguousarray(BV.T),
            "diag": np.ascontiguousarray(dgt.transpose(1, 0, 2).reshape(128, S)),
        })
    return in_maps


_cache = {}


def _get_program(repeat=1):
    key = (HPC, repeat)
    if key not in _cache:
        _cache[key] = build(HPC, repeat=repeat)
    return _cache[key]


def kernel(query, key, value, padding_mask):
    nc = _get_program(repeat=int(os.environ.get("KERNEL_REPEAT", "1")))
    in_maps = host_inputs(query, key, value, padding_mask)
    res = run_bass_kernel_spmd(nc, in_maps, core_ids=list(range(NCORES)))
    out = np.empty((B, H, S, D), dtype=np.float32)
    wts = np.empty((B, H, S, S), dtype=np.float32)
    for c in range(NCORES):
        b = c // CORES_PER_B
        h0 = (c % CORES_PER_B) * HPC
        out[b, h0:h0 + HPC] = res.results[c]["o"]
        wts[b, h0:h0 + HPC] = res.results[c]["w"]
    return out, wts
